# revision 1
# baseline (speedup 1.0000x reference)
"""Grouped multi-query attention on 8 trn2 NeuronCores.

Sharding: data-parallel over batch (2) x sequence-blocks (4) -> 8 cores.
Each core computes output rows [nb*512, (nb+1)*512) of batch g = core//4:
  - projects its 512 query rows (q), and the full 2048 keys/values of its
    batch (k/v work duplicated 4x across the batch group; it is small),
  - attention for all 32 query heads over its rows,
  - output projection for its rows.
No collectives; host concatenates the 8 output slabs.

All tensors are staged transposed (d-major) on host so every matmul runs
with the contraction on the partition axis at free-size 512 (full fp32r
rate). Scores are computed transposed [m, n]; softmax denominators come
from an appended ones-column in the attn@v matmul; exp() runs on the
scalar engine into bf16.
"""
import sys
sys.path.insert(0, '/opt/trn_rl_repo')

import numpy as np

D = 2048          # d_model
N = 2048          # sequence length
NB = 512          # per-core block of query rows
B = 2             # batch
KVH = 8           # kv heads
QH = 32           # query heads
DK = 64           # head dim
G = 4             # query heads per kv head
F32 = None        # set after imports
_NC_CACHE = {}


def _q_perm():
    """Permuted q-head order: chunk t = 4*p+g holds head (kv=2p, g) in
    partitions 0-63 and head (kv=2p+1, g) in partitions 64-127, so the
    scores matmul for both heads of a chunk reads the kv-pair tile p of
    kt_all at a matching base partition."""
    perm = []
    for p in range(4):
        for g in range(4):
            perm += list(range((8 * p + g) * 64, (8 * p + g) * 64 + 64))
            perm += list(range((8 * p + 4 + g) * 64, (8 * p + 4 + g) * 64 + 64))
    return np.array(perm, dtype=np.int64)


def _build_nc():
    import concourse.bacc as bacc
    import concourse.mybir as mybir
    from concourse import tile

    F32 = mybir.dt.float32
    F32R = mybir.dt.float32r
    BF16 = mybir.dt.bfloat16

    nc = bacc.Bacc("TRN2", target_bir_lowering=False, debug=False)

    qt = nc.dram_tensor("qt", [D, NB], F32R, kind="ExternalInput").ap()
    kt = nc.dram_tensor("kt", [D, N], F32R, kind="ExternalInput").ap()
    vt = nc.dram_tensor("vt", [D, N], F32R, kind="ExternalInput").ap()
    wqt = nc.dram_tensor("wqt", [D, D], F32R, kind="ExternalInput").ap()
    wkt = nc.dram_tensor("wkt", [D, KVH * DK], F32R, kind="ExternalInput").ap()
    wvt = nc.dram_tensor("wvt", [D, KVH * DK], F32R, kind="ExternalInput").ap()
    wot = nc.dram_tensor("wot", [D, D], F32R, kind="ExternalInput").ap()
    eye = nc.dram_tensor("eye", [128, 128], F32, kind="ExternalInput").ap()
    outt = nc.dram_tensor("outt", [D, NB], F32, kind="ExternalOutput").ap()

    with tile.TileContext(nc) as tc:
        with (
            tc.tile_pool(name="persist", bufs=1) as persist,
            tc.tile_pool(name="stream", bufs=3) as stream,
            tc.tile_pool(name="small", bufs=2) as small,
        ):
            # persistent SBUF
            qt_all = persist.tile([128, 16 * 512], F32R, tag="qt_all")
            kt_all = persist.tile([128, 4 * 2048], F32R, tag="kt_all")
            v_nat = persist.tile([128, 16 * 520], BF16, tag="v_nat")
            ot_all = persist.tile([128, 16 * 512], F32R, tag="ot_all")
            zbias = persist.tile([128, 1], F32, tag="zbias")
            eye_sb = persist.tile([128, 128], F32, tag="eye")
            ones_pb = persist.tile([65, 64], F32R, tag="ones_pb")
            ones_f = persist.tile([65, 64], F32, tag="ones_f")

            nc.vector.memset(ones_f[:], 1.0)
            nc.vector.tensor_copy(ones_pb[:], ones_f[:])
            nc.vector.memset(zbias[:], 0.0)
            nc.sync.dma_start(eye_sb[:], eye[:])
            # ones column after each head's 64 v-columns: view [128, 128, 65]
            ones_view = v_nat[:].rearrange("p (a c) -> p a c", a=128, c=65)
            nc.vector.memset(ones_view[:, :, 64:65], 1.0)

            # ---------------- projections ----------------
            with tc.tile_pool(name="ppsum", bufs=8, space="PSUM") as ppsum:
                # q-projection: qt_all[j, n] = wq[j, :] @ Q[n, :]
                for half in range(2):
                    ps = [ppsum.tile([128, 512], F32, tag="pp", name=f"pp{i}") for i in range(8)]
                    for dc in range(16):
                        wq_t = stream.tile([128, 1024], F32R, tag="w1024")
                        nc.sync.dma_start(
                            wq_t[:], wqt[dc * 128:(dc + 1) * 128,
                                         half * 1024:(half + 1) * 1024])
                        qt_t = stream.tile([128, 512], F32R, tag="r512")
                        nc.sync.dma_start(qt_t[:], qt[dc * 128:(dc + 1) * 128, :])
                        for j8 in range(8):
                            nc.tensor.matmul(
                                ps[j8][:], wq_t[:, j8 * 128:(j8 + 1) * 128],
                                qt_t[:], start=(dc == 0), stop=(dc == 15))
                    for j8 in range(8):
                        jc = half * 8 + j8
                        nc.vector.tensor_copy(
                            qt_all[:, jc * 512:(jc + 1) * 512], ps[j8][:])

                # k-projection: kt_all[j, m], 4 j-chunks x 2048 m
                for mhalf in range(2):
                    ps = [ppsum.tile([128, 512], F32, tag="pp", name=f"pp{i}") for i in range(8)]
                    for dc in range(16):
                        kt_t = stream.tile([128, 1024], F32R, tag="r1024")
                        nc.sync.dma_start(
                            kt_t[:], kt[dc * 128:(dc + 1) * 128,
                                        mhalf * 1024:(mhalf + 1) * 1024])
                        wk_t = stream.tile([128, 512], F32R, tag="w512")
                        nc.sync.dma_start(wk_t[:], wkt[dc * 128:(dc + 1) * 128, :])
                        for jc in range(4):
                            for mq in range(2):
                                nc.tensor.matmul(
                                    ps[jc * 2 + mq][:],
                                    wk_t[:, jc * 128:(jc + 1) * 128],
                                    kt_t[:, mq * 512:(mq + 1) * 512],
                                    start=(dc == 0), stop=(dc == 15))
                    for jc in range(4):
                        for mq in range(2):
                            m0 = jc * 2048 + mhalf * 1024 + mq * 512
                            nc.vector.tensor_copy(
                                kt_all[:, m0:m0 + 512], ps[jc * 2 + mq][:])

                # v-projection (transposed) then PE-transpose into v_nat
                for mhalf in range(2):
                    ps = [ppsum.tile([128, 512], F32, tag="pp", name=f"pp{i}") for i in range(8)]
                    for dc in range(16):
                        vt_t = stream.tile([128, 1024], F32R, tag="r1024")
                        nc.sync.dma_start(
                            vt_t[:], vt[dc * 128:(dc + 1) * 128,
                                        mhalf * 1024:(mhalf + 1) * 1024])
                        wv_t = stream.tile([128, 512], F32R, tag="w512")
                        nc.sync.dma_start(wv_t[:], wvt[dc * 128:(dc + 1) * 128, :])
                        for jc in range(4):
                            for mq in range(2):
                                nc.tensor.matmul(
                                    ps[jc * 2 + mq][:],
                                    wv_t[:, jc * 128:(jc + 1) * 128],
                                    vt_t[:, mq * 512:(mq + 1) * 512],
                                    start=(dc == 0), stop=(dc == 15))
                    # vT chunk [128 j, 512 m] -> copy to sbuf, transpose 128x128
                    # blocks, write [m, j] into v_nat as bf16
                    for jc in range(4):
                        for mq in range(2):
                            vtmp = small.tile([128, 512], F32, tag="vtmp")
                            nc.vector.tensor_copy(vtmp[:], ps[jc * 2 + mq][:])
                            trp = ppsum.tile([128, 512], F32, tag="pp")
                            for q in range(4):
                                nc.tensor.transpose(
                                    trp[:, q * 128:(q + 1) * 128],
                                    vtmp[:, q * 128:(q + 1) * 128], eye_sb[:])
                            for q in range(4):
                                mc = mhalf * 8 + mq * 4 + q
                                base = mc * 520
                                nc.vector.tensor_copy(
                                    v_nat[:, base + (2 * jc) * 65:
                                          base + (2 * jc) * 65 + 64],
                                    trp[:, q * 128:q * 128 + 64])
                                nc.vector.tensor_copy(
                                    v_nat[:, base + (2 * jc + 1) * 65:
                                          base + (2 * jc + 1) * 65 + 64],
                                    trp[:, q * 128 + 64:q * 128 + 128])

            # ---------------- attention ----------------
            with (
                tc.tile_pool(name="scpsum", bufs=2, space="PSUM") as scpsum,
                tc.tile_pool(name="popsum", bufs=2, space="PSUM") as popsum,
                tc.tile_pool(name="expp", bufs=2) as expp,
            ):
                for h in range(QH):
                    t, b = h // 2, h % 2
                    p = t // 4
                    kh = 2 * p + b       # kv head index
                    qh_ap = qt_all[b * 64:(b + 1) * 64, t * 512:(t + 1) * 512]
                    expT = expp.tile([128, 16 * 512], BF16, tag="expT")
                    for mp in range(8):  # pairs of 128-row m-chunks
                        sc = scpsum.tile([128, 1024], F32, tag="sc")
                        for i in range(2):
                            mc = 2 * mp + i
                            lhs = kt_all[b * 64:(b + 1) * 64,
                                         p * 2048 + mc * 128:
                                         p * 2048 + (mc + 1) * 128]
                            nc.tensor.matmul(
                                sc[:, i * 512:(i + 1) * 512], lhs, qh_ap,
                                start=True, stop=True)
                        nc.scalar.activation(
                            expT[:, mp * 1024:(mp + 1) * 1024], sc[:],
                            mybir.ActivationFunctionType.Exp, bias=zbias[:])
                    po = popsum.tile([65, 512], F32, tag="po")
                    for mc in range(16):
                        nc.tensor.matmul(
                            po[:],
                            v_nat[:, mc * 520 + kh * 65:mc * 520 + kh * 65 + 65],
                            expT[:, mc * 512:(mc + 1) * 512],
                            start=(mc == 0), stop=(mc == 15))
                    rcp = small.tile([65, 512], F32, tag="rcp")
                    nc.vector.reciprocal(rcp[64:65, :], po[64:65, :])
                    rcr = small.tile([65, 512], F32R, tag="rcr")
                    nc.vector.tensor_copy(rcr[64:65, :], rcp[64:65, :])
                    pb = popsum.tile([64, 512], F32, tag="pb")
                    nc.tensor.matmul(pb[:], ones_pb[64:65, :], rcr[64:65, :],
                                     start=True, stop=True)
                    bc = small.tile([64, 512], F32, tag="bc")
                    nc.vector.tensor_copy(bc[:], pb[:])
                    if b == 0:
                        nc.vector.tensor_mul(
                            ot_all[0:64, t * 512:(t + 1) * 512],
                            po[0:64, :], bc[:])
                    else:
                        tmp = small.tile([64, 512], F32R, tag="tmp")
                        nc.vector.tensor_mul(tmp[:], po[0:64, :], bc[:])
                        nc.sync.dma_start(
                            ot_all[64:128, t * 512:(t + 1) * 512], tmp[:])

            # ---------------- output projection ----------------
            with tc.tile_pool(name="opsum", bufs=8, space="PSUM") as opsum:
                for jg in range(2):
                    ps = [opsum.tile([128, 512], F32, tag="op", name=f"op{i}") for i in range(8)]
                    for t in range(16):
                        wo_t = stream.tile([128, 1024], F32R, tag="w1024")
                        nc.sync.dma_start(
                            wo_t[:], wot[t * 128:(t + 1) * 128,
                                         jg * 1024:(jg + 1) * 1024])
                        for j8 in range(8):
                            nc.tensor.matmul(
                                ps[j8][:], wo_t[:, j8 * 128:(j8 + 1) * 128],
                                ot_all[:, t * 512:(t + 1) * 512],
                                start=(t == 0), stop=(t == 15))
                    for j8 in range(8):
                        jc = jg * 8 + j8
                        ostage = stream.tile([128, 512], F32, tag="ostage")
                        nc.vector.tensor_copy(ostage[:], ps[j8][:])
                        nc.sync.dma_start(outt[jc * 128:(jc + 1) * 128, :],
                                          ostage[:])
    nc.compile()
    return nc


def get_nc():
    if "nc" not in _NC_CACHE:
        _NC_CACHE["nc"] = _build_nc()
    return _NC_CACHE["nc"]


def make_in_maps(Q, K, V, w_q, w_k, w_v, w_o):
    perm = _q_perm()
    scale = 1.0 / np.sqrt(DK)
    wqt_p = np.ascontiguousarray((w_q[perm, :] * scale).T.astype(np.float32))
    wkt = np.ascontiguousarray(w_k.T.astype(np.float32))
    wvt = np.ascontiguousarray(w_v.T.astype(np.float32))
    wot_p = np.ascontiguousarray(w_o[:, perm].T.astype(np.float32))
    eye = np.eye(128, dtype=np.float32)
    kts = [np.ascontiguousarray(K[g].T.astype(np.float32)) for g in range(B)]
    vts = [np.ascontiguousarray(V[g].T.astype(np.float32)) for g in range(B)]
    in_maps = []
    for c in range(8):
        g, nb = c // 4, c % 4
        qt_c = np.ascontiguousarray(
            Q[g][nb * NB:(nb + 1) * NB, :].T.astype(np.float32))
        in_maps.append({
            "qt": qt_c, "kt": kts[g], "vt": vts[g],
            "wqt": wqt_p, "wkt": wkt, "wvt": wvt, "wot": wot_p, "eye": eye,
        })
    return in_maps


def kernel(Q, K, V, w_q, w_k, w_v, w_o, b_o):
    from concourse.bass_utils import run_bass_kernel_spmd
    nc = get_nc()
    in_maps = make_in_maps(Q, K, V, w_q, w_k, w_v, w_o)
    res = run_bass_kernel_spmd(nc, in_maps, core_ids=list(range(8)))
    out = np.empty((B, N, D), dtype=np.float32)
    for c in range(8):
        g, nb = c // 4, c % 4
        out[g, nb * NB:(nb + 1) * NB, :] = res.results[c]["outt"].T
    out += np.asarray(b_o, dtype=np.float32)[None, None, :]
    return out

